# revision 11
# baseline (speedup 1.0000x reference)
"""Blocksparse dilated attention TRN2 kernel.

Sharding: 8 cores = r(=4 dilation offsets) x B(=2 batch). Each core runs one
independent per-offset attention branch on its strided token subset
(x[b, o::r, :]), with that offset's own weights. Host does the strided
gather (+transpose to channel-major) and the final scatter into the
zero-padded (B, S, r*D) output.

Per-core math (L=2048 tokens, D=768, H=12 heads, hd=64, segment=512):
  qkvT = Wqkv @ xoT            (channel-on-partition for q,k; token-major v)
  per (segment, head):  scoresT = kT-chunks.T x qT   (k on partitions)
                        attnT = exp(scale * scoresT)  (no max-subtract:
                              scores std ~0.3, max ~1.5 -> exp safe in fp32)
                        [ctxu; denom] = [v | ones].T @ attnT  (ones column
                              makes PSUM row 64 the softmax denominator)
  per segment (batched over heads, off the PE critical path):
                        rc = 1/denom  (one DVE reciprocal for many heads;
                              per-head reciprocals measured 3.35us each and
                              stalled the PE into HAM re-throttle)
                        ctxT = ctxu * broadcast(rc)  (rc staged to DRAM,
                              then partition-step-0 broadcast DMA per chunk)
  outT = Wout @ ctxT + bout

Matmuls run in bf16 (full PE rate; fp32 PSUM accumulation).

DMA strategy (descriptor-generation on a HWDGE queue costs ~0.6us per
128-partition tile, serially): q,k weights go on the scalar-engine HWDGE
queue (idle at startup) in ec-major single-group DMAs (host pre-lays
wqkT so one [128,768] DMA = one full contraction group); xo is loaded
once up-front (segment 0 slice first) on the sync queue, then v weights,
xo remainder, wout. Biases ride the gpsimd SWDGE queue.

Tail: the final segment processes chunks in order [1,2,3,4,5,0] so the
softmax reciprocal for all but the last chunk's 2 heads is done as
attention filler; the output projection accumulates chunks 1..5 into an
SBUF partial during attention, leaving only exp->ctx->recip(2 heads)->
bcast->mul->1-chunk completion after the last scores. Final out DMAs
split across the sync and scalar HWDGE queues.

Emission order software-pipelines segments so the PE never idles long
enough (~3.4us) for the HAM clock gate to drop it from 2.4 to 1.2 GHz.
"""

import math
import sys
from contextlib import ExitStack

import ml_dtypes
import numpy as np

for _p in ("/opt/trn_rl_repo",):
    if _p not in sys.path:
        sys.path.insert(0, _p)

import concourse.bass as bass
import concourse.mybir as mybir
import concourse.tile as tile
from concourse import bacc
from concourse.bass_utils import run_bass_kernel_spmd

P = 128

# Problem constants (hardcoded per harness contract)
B0, S0, D0 = 2, 8192, 768
R0 = 4
H0, HD0 = 12, 64
SEG0 = 512
NSEG0 = (S0 // R0) // SEG0  # 4
N_CORES = 8

F32 = mybir.dt.float32
F32R = mybir.dt.float32r
BF16 = mybir.dt.bfloat16


def build_nc(D=D0, H=H0, HD=HD0, SEG=SEG0, NSEG=NSEG0, mm_dt=BF16):
    """Build the per-core Bass program (same NEFF on all cores)."""
    DC = D // P                # channel chunks of 128
    L = SEG * NSEG             # tokens per core
    KC = SEG // P              # key chunks per segment
    HPC = P // HD              # heads per 128-channel chunk
    E3 = 3 * D
    HV = HD + 1                # v columns per head incl. ones column
    scale = 1.0 / math.sqrt(HD)
    assert D == H * HD and SEG % P == 0 and D % P == 0 and KC % 2 == 0

    nc = bacc.Bacc(trn_type="TRN2")
    xoT = nc.dram_tensor("xoT", [D, L], mm_dt, kind="ExternalInput")
    # q,k weights in ec-major groups: row block g = sec*DC+ec holds
    # [128 part(p), DC*P] where col dc*P+j = W[sec*D+ec*P+j, dc*P+p]
    wqkT = nc.dram_tensor("wqkT", [2 * DC * P, D], mm_dt, kind="ExternalInput")
    wvT = nc.dram_tensor("wvT", [D, D], mm_dt, kind="ExternalInput")
    woutT = nc.dram_tensor("woutT", [D, D], mm_dt, kind="ExternalInput")
    bqkv_pt = nc.dram_tensor("bqkv_pt", [P, 3 * DC], F32, kind="ExternalInput")
    bout_pt = nc.dram_tensor("bout_pt", [P, DC], F32, kind="ExternalInput")
    bv = nc.dram_tensor("bv", [D], F32, kind="ExternalInput")
    outT = nc.dram_tensor("outT", [D, L], mm_dt, kind="ExternalOutput")
    # scratch for the softmax reciprocals: broadcast-DMA needs a DRAM source
    # (SBUF-source partition-step-0 APs are rejected)
    rc_dram = nc.dram_tensor("rc_dram", [NSEG, H * SEG], mm_dt, kind="Internal")

    with ExitStack() as ctx:
        tc = ctx.enter_context(tile.TileContext(nc))
        singles = ctx.enter_context(tc.tile_pool(name="singles", bufs=1))
        qk_pool = ctx.enter_context(tc.tile_pool(name="qk", bufs=1))
        v_pool = ctx.enter_context(tc.tile_pool(name="v", bufs=1))
        attn_pool = ctx.enter_context(tc.tile_pool(name="attn", bufs=3))
        ctxu_pool = ctx.enter_context(tc.tile_pool(name="ctxu", bufs=2))
        den_pool = ctx.enter_context(tc.tile_pool(name="den", bufs=2))
        ctxs_pool = ctx.enter_context(tc.tile_pool(name="ctxs", bufs=2))
        out_pool = ctx.enter_context(tc.tile_pool(name="outp", bufs=4))
        bcast_pool = ctx.enter_context(tc.tile_pool(name="bcast", bufs=4))
        pp_proj = ctx.enter_context(tc.tile_pool(name="pp_proj", bufs=2, space="PSUM"))
        pp_scA = ctx.enter_context(tc.tile_pool(name="pp_scA", bufs=1, space="PSUM"))
        pp_scB = ctx.enter_context(tc.tile_pool(name="pp_scB", bufs=1, space="PSUM"))
        pp_cb = ctx.enter_context(tc.tile_pool(name="pp_cb", bufs=2, space="PSUM"))

        # --- resident xo: segment-0 slice first (gates the first matmul
        # group), remainder later on the sync queue ---
        xo_sb = singles.tile([P, DC, L], mm_dt, tag="xo")
        for dc in range(DC):
            nc.sync.dma_start(out=xo_sb[:, dc, 0:SEG],
                              in_=xoT[dc * P:(dc + 1) * P, 0:SEG])

        # --- q,k weights on the scalar HWDGE queue: one DMA per ec group
        # (host supplies dc-concatenated lines) ---
        w_qkv_sb = singles.tile([P, DC, E3], mm_dt, tag="wqkv")
        for sec in range(2):
            for ec in range(DC):
                g = sec * DC + ec
                nc.scalar.dma_start(
                    out=w_qkv_sb[:, :, sec * D + ec * P: sec * D + (ec + 1) * P],
                    in_=wqkT[g * P:(g + 1) * P, :])

        # --- biases on the gpsimd SWDGE queue (desc-gen for the tiny
        # [128,18] tiles costs >1us on the HWDGE queues) ---
        bqkv_sb = singles.tile([P, 3 * DC], F32, tag="bqkv")
        nc.gpsimd.dma_start(out=bqkv_sb, in_=bqkv_pt[:, :])
        bout_sb = singles.tile([P, DC], F32, tag="bout")
        nc.gpsimd.dma_start(out=bout_sb, in_=bout_pt[:, :])
        # v-section bias broadcast along partitions (natural layout add)
        bv_sb = singles.tile([P, D], F32, tag="bv")
        bv_ap = bv[:]
        bv_bcast = bass.AP(tensor=bv_ap.tensor, offset=bv_ap.offset,
                           ap=[[0, P], *bv_ap.ap])
        nc.gpsimd.dma_start(out=bv_sb, in_=bv_bcast)

        # --- rest of the sync queue: v weights (needed ~ctx(0)), xo
        # remainder (needed ~proj(1)), wout (needed ~attention(1)) ---
        for dc in range(DC):
            nc.sync.dma_start(out=w_qkv_sb[:, dc, 2 * D:3 * D],
                              in_=wvT[dc * P:(dc + 1) * P, :])
        for dc in range(DC):
            nc.sync.dma_start(out=xo_sb[:, dc, SEG:L],
                              in_=xoT[dc * P:(dc + 1) * P, SEG:L])
        w_out_sb = singles.tile([P, DC, D], mm_dt, tag="wout")
        for dc in range(DC):
            nc.sync.dma_start(out=w_out_sb[:, dc, :],
                              in_=woutT[dc * P:(dc + 1) * P, :])

        def load_and_proj(s):
            """qkv projections for segment s (xo already resident)."""
            st = {}
            o0 = s * SEG
            st["ctxu"] = ctxu_pool.tile([P, DC, SEG], F32, tag="ctxu",
                                        name=f"ctxu{s}")
            st["den"] = den_pool.tile([1, H * SEG], F32, tag="den",
                                      name=f"den{s}")
            st["ctx_s"] = ctxs_pool.tile([P, DC, SEG], mm_dt, tag="ctxs",
                                         name=f"cs{s}")

            # q,k in transposed layout (e on partitions)
            qk_s = qk_pool.tile([P, 2 * DC, SEG], mm_dt, tag="qk", name=f"qk_s{s}")
            st["qk"] = qk_s
            for ec in range(2 * DC):
                ps = pp_proj.tile([P, SEG], F32, tag="proj", name=f"psqk{s}_{ec}")
                for dc in range(DC):
                    nc.tensor.matmul(
                        ps,
                        w_qkv_sb[:, dc, ec * P:(ec + 1) * P],
                        xo_sb[:, dc, o0:o0 + SEG],
                        start=(dc == 0), stop=(dc == DC - 1))
                nc.vector.tensor_scalar_add(qk_s[:, ec, :], ps, bqkv_sb[:, ec:ec + 1])

            # v in natural layout (token on partitions), per-head + ones column
            v_s = v_pool.tile([P, KC, H * HV], mm_dt, tag="v", name=f"v_s{s}")
            st["v"] = v_s
            v_view = v_s.rearrange("p k (h c) -> p k h c", c=HV)
            nc.vector.memset(v_view[:, :, :, HD:HD + 1], 1.0)
            for lc in range(KC):
                for n0 in range(0, D, 512):
                    n = min(512, D - n0)
                    nh = n // HD
                    h0 = n0 // HD
                    psv = pp_proj.tile([P, SEG], F32, tag="proj",
                                       name=f"psv{s}_{lc}_{n0}")
                    for dc in range(DC):
                        nc.tensor.matmul(
                            psv[:, :n],
                            xo_sb[:, dc, o0 + lc * P:o0 + (lc + 1) * P],
                            w_qkv_sb[:, dc, 2 * D + n0: 2 * D + n0 + n],
                            start=(dc == 0), stop=(dc == DC - 1))
                    nc.vector.tensor_add(
                        v_view[:, lc, h0:h0 + nh, 0:HD],
                        psv[:, :n].rearrange("p (h c) -> p h c", c=HD),
                        bv_sb[:, n0:n0 + n].rearrange("p (h c) -> p h c", c=HD))
            return st

        def attention(s, st, filler=(), cs=None, filler_at=None):
            """scores + exp + unnormalized ctx (and denom), processed in
            head PAIRS: the two heads of a 128-channel chunk occupy PE
            row-groups 0-63 and 64-127, and their K=64 scores matmuls are
            emitted adjacently so the array runs them concurrently (~2x on
            the scores phase). Pipelined: ctx(pair-1) after scores(pair).
            `filler` tasks (prev segment's normalize + outproj) are emitted
            between pairs so the PE has work while ACT exp catches up.
            `cs` optionally permutes the chunk processing order; `filler_at`
            ({iteration: [units]}) pins units to iterations (emission-order
            matters: a unit must be emitted after its producers)."""
            if cs is None:
                cs = list(range(DC))
            filler = list(filler)
            n_filler = len(filler)
            filler_at = filler_at or {}
            emitted = 0
            qk_s, v_s = st["qk"], st["v"]
            ctxu, den = st["ctxu"], st["den"]
            ats = {}
            for i in range(DC + 1):
                while emitted < (i * n_filler) // DC:
                    filler[emitted]()
                    emitted += 1
                if i < DC:
                    c = cs[i]
                    at2 = attn_pool.tile([P, HPC, KC, SEG], mm_dt, tag="attn",
                                         name=f"at{s}_{c}")
                    ats[c] = at2
                    for w in range(KC // 2):
                        for half, pool in ((0, pp_scA), (1, pp_scB)):
                            kc = 2 * w + half
                            sc = pool.tile([P, HPC, SEG], F32, tag=f"sc{half}",
                                           name=f"sc{half}_{s}_{c}_{w}")
                            for hi in range(HPC):
                                ho = hi * HD
                                nc.tensor.matmul(
                                    sc[:, hi, :],
                                    qk_s[ho:ho + HD, DC + c, kc * P:(kc + 1) * P],
                                    qk_s[ho:ho + HD, c, :])
                            nc.scalar.activation(
                                at2[:, :, kc, :], sc,
                                mybir.ActivationFunctionType.Exp,
                                scale=scale)
                if i > 0:
                    cp = cs[i - 1]
                    at2 = ats.pop(cp)
                    for hi in range(HPC):
                        h = cp * HPC + hi
                        ho = hi * HD
                        cps = pp_cb.tile([HD + 1, SEG], F32, tag="cb",
                                         name=f"cps{s}_{h}")
                        for kc in range(KC):
                            nc.tensor.matmul(
                                cps,
                                v_s[:, kc, h * HV:(h + 1) * HV],
                                at2[:, hi, kc, :],
                                start=(kc == 0), stop=(kc == KC - 1))
                        nc.vector.tensor_copy(ctxu[ho:ho + HD, cp, :],
                                              cps[0:HD, :])
                        nc.vector.tensor_copy(den[0:1, h * SEG:(h + 1) * SEG],
                                              cps[HD:HD + 1, :])
                # pinned units run at the BOTTOM of the iteration, after
                # ctx(cs[i-1])'s den copies (their usual producers), so a
                # gated unit never head-of-line-blocks this iteration's DVE
                for task in filler_at.get(i, ()):
                    task()
            while emitted < n_filler:
                filler[emitted]()
                emitted += 1

        def recip_chain(s, st, h0=0, nh=H):
            """Reciprocal of the softmax denominators for heads [h0, h0+nh)
            (DMA/DVE only, no PE). DVE reciprocal costs ~6.5ns/element/lane,
            so a single-partition strip would take ~40us: round-trip a DMA
            "transpose" to spread the elements over all 128 partitions
            (element order irrelevant: reciprocal is elementwise and the
            second DMA restores order)."""
            den = st["den"]
            e0, ne = h0 * SEG, nh * SEG
            assert ne % P == 0
            den_t = den_pool.tile([P, ne // P], F32, tag="dent",
                                  name=f"dent{s}_{h0}")
            nc.scalar.dma_start(out=den_t, in_=den[0:1, e0:e0 + ne])
            rc_t = den_pool.tile([P, ne // P], mm_dt, tag="rct",
                                 name=f"rct{s}_{h0}")
            with nc.allow_low_precision(
                    reason="softmax denominator reciprocal; bf16 scale factor"):
                nc.vector.reciprocal(rc_t, den_t)
            nc.scalar.dma_start(out=rc_dram[s:s + 1, e0:e0 + ne], in_=rc_t)

        def norm_tasks(s, st, hc0=0, hc1=None, full=True):
            """Deferred normalize + outproj tasks (run as PE/DVE filler inside
            the next segment's attention). The per-head reciprocal row is
            broadcast across HD partitions by an SWDGE DMA (partition-step-0
            source AP) instead of a PE outer-product matmul."""
            if hc1 is None:
                hc1 = DC
            ctxu, ctx_s = st["ctxu"], st["ctx_s"]

            def norm_chunk(hc):
                # broadcast the HPC reciprocal rows of this head-chunk into a
                # full 128-partition tile (walrus requires equal base
                # partitions when both TensorTensor inputs are in SBUF)
                bcs = bcast_pool.tile([P, SEG], mm_dt, tag="bcs",
                                      name=f"bcs{s}_{hc}")
                rr = rc_dram[s:s + 1, hc * HPC * SEG:(hc + 1) * HPC * SEG]
                rr_b = bass.AP(tensor=rr.tensor, offset=rr.offset,
                               ap=[[SEG, HPC], [0, HD], [1, SEG]])
                nc.sync.dma_start(out=bcs, in_=rr_b)
                nc.vector.tensor_mul(ctx_s[:, hc, :], ctxu[:, hc, :], bcs)

            def outproj(fc):
                pso = pp_proj.tile([P, SEG], F32, tag="proj", name=f"pso{s}_{fc}")
                for dc in range(DC):
                    nc.tensor.matmul(
                        pso,
                        w_out_sb[:, dc, fc * P:(fc + 1) * P],
                        ctx_s[:, dc, :],
                        start=(dc == 0), stop=(dc == DC - 1))
                ot = out_pool.tile([P, SEG], mm_dt, tag="ot", name=f"ot{s}_{fc}")
                nc.vector.tensor_scalar_add(ot, pso, bout_sb[:, fc:fc + 1])
                nc.sync.dma_start(
                    out=outT[fc * P:(fc + 1) * P, s * SEG:(s + 1) * SEG], in_=ot)

            return ([(lambda hc=hc: norm_chunk(hc)) for hc in range(hc0, hc1)]
                    + [(lambda fc=fc: outproj(fc)) for fc in range(DC)]
                    if full else
                    [(lambda hc=hc: norm_chunk(hc)) for hc in range(hc0, hc1)])

        sts = {}
        SL = NSEG - 1
        for s in range(NSEG):
            sts[s] = load_and_proj(s)
            if s < SL:
                filler = list(norm_tasks(s - 1, sts.pop(s - 1))) if s > 0 else []
                attention(s, sts[s], filler)
                recip_chain(s, sts[s])

        # --- final segment: chunk order [1..5, 0]; recip + normalize +
        # partial outproj (chunks 1..5) all run as attention filler, so
        # after the last ctx only a 2-head recip + 1-chunk completion
        # remains ---
        stl = sts[SL]
        partial = ctxu_pool.tile([P, DC, SEG], mm_dt, tag="partial",
                                 name="partial", bufs=1)

        def partial_outproj(fc):
            pso = pp_proj.tile([P, SEG], F32, tag="proj", name=f"ppo{fc}")
            for dc in range(1, DC):
                nc.tensor.matmul(
                    pso,
                    w_out_sb[:, dc, fc * P:(fc + 1) * P],
                    stl["ctx_s"][:, dc, :],
                    start=(dc == 1), stop=(dc == DC - 1))
            nc.vector.tensor_scalar_add(partial[:, fc, :], pso,
                                        bout_sb[:, fc:fc + 1])

        prev_tasks = list(norm_tasks(SL - 1, sts.pop(SL - 1)))
        nrm = norm_tasks(SL, stl, hc0=1, hc1=DC, full=False)  # hc1..hc5 units
        # pin recip/normalize/partial units to iteration BOTTOMS so each is
        # emitted right after the ctx that produces its denominators (chunk
        # order [1..5,0]: ctx(cs[i-1]) lands at the bottom of iteration i)
        filler_at = {
            2: [lambda: recip_chain(SL, stl, 2, 4)],
            3: [nrm[0], nrm[1]],
            4: [lambda: recip_chain(SL, stl, 6, 4)],
            5: [lambda: recip_chain(SL, stl, 10, 2), nrm[2], nrm[3]],
            6: ([lambda: recip_chain(SL, stl, 0, 2), nrm[4]]
                + [(lambda fc=fc: partial_outproj(fc)) for fc in range(DC)]),
        }
        attention(SL, stl, prev_tasks, cs=[1, 2, 3, 4, 5, 0],
                  filler_at=filler_at)

        # endgame: heads 0,1 only (reciprocal already issued at post[6])
        bcs0 = bcast_pool.tile([P, SEG], mm_dt, tag="bcs", name="ebc0")
        rr = rc_dram[SL:SL + 1, 0:HPC * SEG]
        rr_b = bass.AP(tensor=rr.tensor, offset=rr.offset,
                       ap=[[SEG, HPC], [0, HD], [1, SEG]])
        nc.sync.dma_start(out=bcs0, in_=rr_b)
        nc.vector.tensor_mul(stl["ctx_s"][:, 0, :], stl["ctxu"][:, 0, :], bcs0)
        for fc in range(DC):
            pso = pp_proj.tile([P, SEG], F32, tag="proj", name=f"cmp{fc}")
            nc.tensor.matmul(
                pso,
                w_out_sb[:, 0, fc * P:(fc + 1) * P],
                stl["ctx_s"][:, 0, :],
                start=True, stop=True)
            ot = out_pool.tile([P, SEG], mm_dt, tag="ot", name=f"eo{fc}")
            nc.vector.tensor_add(ot, partial[:, fc, :], pso)
            eng = nc.sync if fc % 2 == 0 else nc.scalar
            eng.dma_start(
                out=outT[fc * P:(fc + 1) * P, SL * SEG:(SL + 1) * SEG],
                in_=ot)

    nc.compile()
    return nc


def make_in_maps(x, Wqkv, bqkv, Wout, bout):
    """Shard full inputs across 8 cores: core = o*B + b."""
    r, E3, D = Wqkv.shape
    Bb, S, _ = x.shape
    DC = D // P
    in_maps = []
    for c in range(r * Bb):
        o, b = c // Bb, c % Bb
        W = np.ascontiguousarray(Wqkv[o])
        # ec-major q,k layout: row block g=sec*DC+ec is [128(p), DC*P] with
        # col dc*P+j = W[sec*D+ec*P+j, dc*P+p]
        wqk = W[:2 * D].reshape(2, DC, P, DC, P).transpose(0, 1, 4, 3, 2)
        wqkT_h = np.ascontiguousarray(wqk.reshape(2 * DC * P, D))
        in_maps.append({
            "xoT": np.ascontiguousarray(x[b, o::r, :].T).astype(ml_dtypes.bfloat16),
            "wqkT": wqkT_h.astype(ml_dtypes.bfloat16),
            "wvT": np.ascontiguousarray(W[2 * D:].T).astype(ml_dtypes.bfloat16),
            "woutT": np.ascontiguousarray(Wout[o].T).astype(ml_dtypes.bfloat16),
            "bqkv_pt": np.ascontiguousarray(bqkv[o].reshape(3 * DC, P).T),
            "bout_pt": np.ascontiguousarray(bout[o].reshape(DC, P).T),
            "bv": np.ascontiguousarray(bqkv[o, 2 * D:3 * D]),
        })
    return in_maps


_NC_CACHE = {}


def get_nc():
    if "nc" not in _NC_CACHE:
        _NC_CACHE["nc"] = build_nc()
    return _NC_CACHE["nc"]


def run(inputs, trace=False, **kwargs):
    """Run the SPMD kernel; returns (full_output, BassKernelResults)."""
    x = np.ascontiguousarray(np.asarray(inputs["x"], dtype=np.float32))
    Wqkv = np.asarray(inputs["Wqkv"], dtype=np.float32)
    bqkv = np.asarray(inputs["bqkv"], dtype=np.float32)
    Wout = np.asarray(inputs["Wout"], dtype=np.float32)
    bout = np.asarray(inputs["bout"], dtype=np.float32)
    r, E3, D = Wqkv.shape
    Bb, S, _ = x.shape

    nc = get_nc()
    in_maps = make_in_maps(x, Wqkv, bqkv, Wout, bout)
    res = run_bass_kernel_spmd(nc, in_maps, core_ids=list(range(len(in_maps))),
                               trace=trace, **kwargs)

    out = np.zeros((Bb, S, r * D), np.float32)
    for c in range(len(in_maps)):
        o, b = c // Bb, c % Bb
        out[b, o::r, o * D:(o + 1) * D] = \
            np.asarray(res.results[c]["outT"]).astype(np.float32).T
    return out, res


def kernel(x, Wqkv, bqkv, Wout, bout, num_heads):
    assert int(num_heads) == H0
    out, _ = run(dict(x=x, Wqkv=Wqkv, bqkv=bqkv, Wout=Wout, bout=bout))
    return out


# revision 30
# speedup vs baseline: 1.0123x; 1.0123x over previous
"""Blocksparse dilated attention TRN2 kernel.

Sharding: 8 cores = r(=4 dilation offsets) x B(=2 batch). Each core runs one
independent per-offset attention branch on its strided token subset
(x[b, o::r, :]), with that offset's own weights. Host does the strided
gather (+transpose to channel-major) and the final scatter into the
zero-padded (B, S, r*D) output.

Per-core math (L=2048 tokens, D=768, H=12 heads, hd=64, segment=512):
  qkvT = Wqkv @ xoT            (channel-on-partition for q,k; token-major v)
  per (segment, head):  scoresT = kT-chunks.T x qT   (k on partitions)
                        attnT = exp(scale * scoresT)  (no max-subtract:
                              scores std ~0.3, max ~1.5 -> exp safe in fp32)
                        [ctxu; denom] = [v | ones].T @ attnT  (ones column
                              makes PSUM row 64 the softmax denominator)
  per segment (batched over heads, off the PE critical path):
                        rc = 1/denom  (one DVE reciprocal for many heads;
                              per-head reciprocals measured 3.35us each and
                              stalled the PE into HAM re-throttle)
                        ctxT = ctxu * broadcast(rc)  (rc staged to DRAM,
                              then partition-step-0 broadcast DMA per chunk)
  outT = Wout @ ctxT + bout

Matmuls run in bf16 (full PE rate; fp32 PSUM accumulation).

DMA strategy (descriptor-generation on a HWDGE queue costs ~0.6us per
128-partition tile, serially): q,k weights go on the scalar-engine HWDGE
queue (idle at startup) in ec-major single-group DMAs (host pre-lays
wqkT so one [128,768] DMA = one full contraction group); xo is loaded
once up-front (segment 0 slice first) on the sync queue, then v weights,
xo remainder, wout. Biases ride the gpsimd SWDGE queue.

Tail: the final segment processes chunks in order [1,2,3,4,5,0] so the
softmax reciprocal for all but the last chunk's 2 heads is done as
attention filler; the output projection accumulates chunks 1..5 into an
SBUF partial during attention, leaving only exp->ctx->recip(2 heads)->
bcast->mul->1-chunk completion after the last scores. Final out DMAs
split across the sync and scalar HWDGE queues.

Emission order software-pipelines segments so the PE never idles long
enough (~3.4us) for the HAM clock gate to drop it from 2.4 to 1.2 GHz.
"""

import math
import sys
from contextlib import ExitStack

import ml_dtypes
import numpy as np

for _p in ("/opt/trn_rl_repo",):
    if _p not in sys.path:
        sys.path.insert(0, _p)

import concourse.bass as bass
import concourse.mybir as mybir
import concourse.tile as tile
from concourse import bacc
from concourse.bass_utils import run_bass_kernel_spmd

P = 128

# Problem constants (hardcoded per harness contract)
B0, S0, D0 = 2, 8192, 768
R0 = 4
H0, HD0 = 12, 64
SEG0 = 512
NSEG0 = (S0 // R0) // SEG0  # 4
N_CORES = 8

F32 = mybir.dt.float32
F32R = mybir.dt.float32r
BF16 = mybir.dt.bfloat16


def build_nc(D=D0, H=H0, HD=HD0, SEG=SEG0, NSEG=NSEG0, mm_dt=BF16):
    """Build the per-core Bass program (same NEFF on all cores)."""
    DC = D // P                # channel chunks of 128
    L = SEG * NSEG             # tokens per core
    KC = SEG // P              # key chunks per segment
    HPC = P // HD              # heads per 128-channel chunk
    E3 = 3 * D
    HV = HD + 1                # v columns per head incl. ones column
    scale = 1.0 / math.sqrt(HD)
    assert D == H * HD and SEG % P == 0 and D % P == 0 and KC % 2 == 0

    nc = bacc.Bacc(trn_type="TRN2")
    xoT = nc.dram_tensor("xoT", [D, L], mm_dt, kind="ExternalInput")
    # q,k weights in ec-major groups: row block g = sec*DC+ec holds
    # [128 part(p), DC*P] where col dc*P+j = W[sec*D+ec*P+j, dc*P+p]
    wqkT = nc.dram_tensor("wqkT", [2 * DC * P, D], mm_dt, kind="ExternalInput")
    wvT = nc.dram_tensor("wvT", [D, D], mm_dt, kind="ExternalInput")
    woutT = nc.dram_tensor("woutT", [D, D], mm_dt, kind="ExternalInput")
    mask2_d = nc.dram_tensor("mask2", [1, HPC * P], mm_dt, kind="ExternalInput")
    bqkv_pt = nc.dram_tensor("bqkv_pt", [P, 3 * DC], F32, kind="ExternalInput")
    bout_pt = nc.dram_tensor("bout_pt", [P, DC], F32, kind="ExternalInput")
    bv = nc.dram_tensor("bv", [D], F32, kind="ExternalInput")
    outT = nc.dram_tensor("outT", [D, L], mm_dt, kind="ExternalOutput")
    # scratch for the softmax reciprocals: broadcast-DMA needs a DRAM source
    # (SBUF-source partition-step-0 APs are rejected)
    rc_dram = nc.dram_tensor("rc_dram", [NSEG, H * SEG], mm_dt, kind="Internal")

    with ExitStack() as ctx:
        tc = ctx.enter_context(tile.TileContext(nc))
        singles = ctx.enter_context(tc.tile_pool(name="singles", bufs=1))
        qk_pool = ctx.enter_context(tc.tile_pool(name="qk", bufs=1))
        v_pool = ctx.enter_context(tc.tile_pool(name="v", bufs=1))
        attn_pool = ctx.enter_context(tc.tile_pool(name="attn", bufs=3))
        ctxu_pool = ctx.enter_context(tc.tile_pool(name="ctxu", bufs=2))
        den_pool = ctx.enter_context(tc.tile_pool(name="den", bufs=2))
        ctxs_pool = ctx.enter_context(tc.tile_pool(name="ctxs", bufs=2))
        out_pool = ctx.enter_context(tc.tile_pool(name="outp", bufs=4))
        bcast_pool = ctx.enter_context(tc.tile_pool(name="bcast", bufs=4))
        pp_proj = ctx.enter_context(tc.tile_pool(name="pp_proj", bufs=2, space="PSUM"))
        pp_scA = ctx.enter_context(tc.tile_pool(name="pp_scA", bufs=1, space="PSUM"))
        pp_scB = ctx.enter_context(tc.tile_pool(name="pp_scB", bufs=1, space="PSUM"))
        pp_cb = ctx.enter_context(tc.tile_pool(name="pp_cb", bufs=2, space="PSUM"))

        # --- startup-critical DMAs interleaved across the two HWDGE queues
        # (desc-gen is ~0.6us per tile DMA, serial per queue): q weights and
        # segment-0 xo alternate so the first accumulation group's inputs
        # stream in dc order on both queues ---
        xo_sb = singles.tile([P, DC, L], mm_dt, tag="xo")
        w_qkv_sb = singles.tile([P, DC, E3], mm_dt, tag="wqkv")

        def wqk_dma(eng, sec, ec):
            g = sec * DC + ec
            eng.dma_start(
                out=w_qkv_sb[:, :, sec * D + ec * P: sec * D + (ec + 1) * P],
                in_=wqkT[g * P:(g + 1) * P, :])

        for dc in range(DC):
            if dc % 2 == 0:
                wqk_dma(nc.sync, 0, dc)
                nc.scalar.dma_start(out=xo_sb[:, dc, 0:SEG],
                                    in_=xoT[dc * P:(dc + 1) * P, 0:SEG])
            else:
                wqk_dma(nc.scalar, 0, dc)
                nc.sync.dma_start(out=xo_sb[:, dc, 0:SEG],
                                  in_=xoT[dc * P:(dc + 1) * P, 0:SEG])
        for ec in range(DC):
            wqk_dma(nc.scalar if ec % 2 == 0 else nc.sync, 1, ec)

        # --- biases on the gpsimd SWDGE queue (desc-gen for the tiny
        # [128,18] tiles costs >1us on the HWDGE queues) ---
        bqkv_sb = singles.tile([P, 3 * DC], F32, tag="bqkv")
        nc.gpsimd.dma_start(out=bqkv_sb, in_=bqkv_pt[:, :])
        bout_sb = singles.tile([P, DC], F32, tag="bout")
        nc.gpsimd.dma_start(out=bout_sb, in_=bout_pt[:, :])
        # v-section bias broadcast along partitions (natural layout add)
        bv_sb = singles.tile([P, D], F32, tag="bv")
        bv_ap = bv[:]
        bv_bcast = bass.AP(tensor=bv_ap.tensor, offset=bv_ap.offset,
                           ap=[[0, P], *bv_ap.ap])
        nc.gpsimd.dma_start(out=bv_sb, in_=bv_bcast)

        # --- rest of the sync queue: v weights (needed ~ctx(0)), xo
        # remainder (needed ~proj(1)), wout (needed ~attention(1)) ---
        for dc in range(DC):
            nc.sync.dma_start(out=w_qkv_sb[:, dc, 2 * D:3 * D],
                              in_=wvT[dc * P:(dc + 1) * P, :])
        for dc in range(DC):
            nc.sync.dma_start(out=xo_sb[:, dc, SEG:L],
                              in_=xoT[dc * P:(dc + 1) * P, SEG:L])
        w_out_sb = singles.tile([P, DC, D], mm_dt, tag="wout")
        for dc in range(DC):
            nc.sync.dma_start(out=w_out_sb[:, dc, :],
                              in_=woutT[dc * P:(dc + 1) * P, :])

        def load_and_proj(s):
            """qkv projections for segment s (xo already resident)."""
            st = {}
            o0 = s * SEG
            st["ctxu"] = ctxu_pool.tile([P, DC, SEG], F32, tag="ctxu",
                                        name=f"ctxu{s}")
            st["den"] = den_pool.tile([1, H * SEG], F32, tag="den",
                                      name=f"den{s}", bufs=1)
            st["ctx_s"] = ctxs_pool.tile([P, DC, SEG], mm_dt, tag="ctxs",
                                         name=f"cs{s}")

            # q,k in transposed layout (e on partitions)
            qk_s = qk_pool.tile([P, 2 * DC, SEG], mm_dt, tag="qk", name=f"qk_s{s}")
            st["qk"] = qk_s
            for ec in range(2 * DC):
                ps = pp_proj.tile([P, SEG], F32, tag="proj", name=f"psqk{s}_{ec}")
                for dc in range(DC):
                    nc.tensor.matmul(
                        ps,
                        w_qkv_sb[:, dc, ec * P:(ec + 1) * P],
                        xo_sb[:, dc, o0:o0 + SEG],
                        start=(dc == 0), stop=(dc == DC - 1))
                nc.vector.tensor_scalar_add(qk_s[:, ec, :], ps, bqkv_sb[:, ec:ec + 1])

            # v in natural layout (token on partitions), per-head + ones column
            v_s = v_pool.tile([P, KC, H * HV], mm_dt, tag="v", name=f"v_s{s}")
            st["v"] = v_s
            v_view = v_s.rearrange("p k (h c) -> p k h c", c=HV)
            nc.vector.memset(v_view[:, :, :, HD:HD + 1], 1.0)
            for lc in range(KC):
                for n0 in range(0, D, 512):
                    n = min(512, D - n0)
                    nh = n // HD
                    h0 = n0 // HD
                    psv = pp_proj.tile([P, SEG], F32, tag="proj",
                                       name=f"psv{s}_{lc}_{n0}")
                    for dc in range(DC):
                        nc.tensor.matmul(
                            psv[:, :n],
                            xo_sb[:, dc, o0 + lc * P:o0 + (lc + 1) * P],
                            w_qkv_sb[:, dc, 2 * D + n0: 2 * D + n0 + n],
                            start=(dc == 0), stop=(dc == DC - 1))
                    nc.vector.tensor_add(
                        v_view[:, lc, h0:h0 + nh, 0:HD],
                        psv[:, :n].rearrange("p (h c) -> p h c", c=HD),
                        bv_sb[:, n0:n0 + n].rearrange("p (h c) -> p h c", c=HD))
            return st

        def attention(s, st, filler=(), cs=None, filler_at=None,
                      den2_chunk=None, den2=None):
            """scores + exp + unnormalized ctx (and denom), processed in
            head PAIRS: the two heads of a 128-channel chunk occupy PE
            row-groups 0-63 and 64-127, and their K=64 scores matmuls are
            emitted adjacently so the array runs them concurrently (~2x on
            the scores phase). Pipelined: ctx(pair-1) after scores(pair).
            `filler` tasks (prev segment's normalize + outproj) are emitted
            between pairs so the PE has work while ACT exp catches up.
            `cs` optionally permutes the chunk processing order; `filler_at`
            ({iteration: [units]}) pins units to iterations (emission-order
            matters: a unit must be emitted after its producers)."""
            if cs is None:
                cs = list(range(DC))
            filler = list(filler)
            n_filler = len(filler)
            filler_at = filler_at or {}
            emitted = 0
            qk_s, v_s = st["qk"], st["v"]
            ctxu, den = st["ctxu"], st["den"]
            ats = {}
            for i in range(DC + 1):
                while emitted < (i * n_filler) // DC:
                    filler[emitted]()
                    emitted += 1
                if i < DC:
                    c = cs[i]
                    at2 = attn_pool.tile([P, HPC, KC, SEG], mm_dt, tag="attn",
                                         name=f"at{s}_{c}")
                    ats[c] = at2
                    for w in range(KC // 2):
                        for half, pool in ((0, pp_scA), (1, pp_scB)):
                            kc = 2 * w + half
                            sc = pool.tile([P, HPC, SEG], F32, tag=f"sc{half}",
                                           name=f"sc{half}_{s}_{c}_{w}")
                            for hi in range(HPC):
                                ho = hi * HD
                                nc.tensor.matmul(
                                    sc[:, hi, :],
                                    qk_s[ho:ho + HD, DC + c, kc * P:(kc + 1) * P],
                                    qk_s[ho:ho + HD, c, :])
                            nc.scalar.activation(
                                at2[:, :, kc, :], sc,
                                mybir.ActivationFunctionType.Exp,
                                scale=scale)
                if i > 0:
                    cp = cs[i - 1]
                    at2 = ats.pop(cp)
                    for hi in range(HPC):
                        h = cp * HPC + hi
                        ho = hi * HD
                        cps = pp_cb.tile([HD + 1, SEG], F32, tag="cb",
                                         name=f"cps{s}_{h}")
                        for kc in range(KC):
                            nc.tensor.matmul(
                                cps,
                                v_s[:, kc, h * HV:(h + 1) * HV],
                                at2[:, hi, kc, :],
                                start=(kc == 0), stop=(kc == KC - 1))
                        nc.vector.tensor_copy(ctxu[ho:ho + HD, cp, :],
                                              cps[0:HD, :])
                        if cp == den2_chunk:
                            # endgame chunk: land the denominators in a
                            # partition-0 row for the fast-reciprocal +
                            # PE-broadcast path (no DRAM round trip)
                            nc.vector.tensor_copy(
                                den2[0:1, hi * SEG:(hi + 1) * SEG],
                                cps[HD:HD + 1, :])
                        else:
                            nc.vector.tensor_copy(
                                den[0:1, h * SEG:(h + 1) * SEG],
                                cps[HD:HD + 1, :])
                # pinned units run at the BOTTOM of the iteration, after
                # ctx(cs[i-1])'s den copies (their usual producers), so a
                # gated unit never head-of-line-blocks this iteration's DVE
                for task in filler_at.get(i, ()):
                    task()
            while emitted < n_filler:
                filler[emitted]()
                emitted += 1

        def recip_chain(s, st, h0=0, nh=H):
            """Reciprocal of the softmax denominators for heads [h0, h0+nh)
            (DMA/DVE only, no PE). DVE reciprocal costs ~6.5ns/element/lane,
            so a single-partition strip would take ~40us: round-trip a DMA
            "transpose" to spread the elements over all 128 partitions
            (element order irrelevant: reciprocal is elementwise and the
            second DMA restores order)."""
            den = st["den"]
            e0, ne = h0 * SEG, nh * SEG
            assert ne % P == 0
            den_t = den_pool.tile([P, ne // P], F32, tag="dent",
                                  name=f"dent{s}_{h0}")
            nc.gpsimd.dma_start(out=den_t, in_=den[0:1, e0:e0 + ne])
            rc_t = den_pool.tile([P, ne // P], mm_dt, tag="rct",
                                 name=f"rct{s}_{h0}")
            with nc.allow_low_precision(
                    reason="softmax denominator reciprocal; bf16 scale factor"):
                nc.vector.reciprocal(rc_t, den_t)
            nc.gpsimd.dma_start(out=rc_dram[s:s + 1, e0:e0 + ne], in_=rc_t)

        def norm_tasks(s, st, hc0=0, hc1=None, full=True):
            """Deferred normalize + outproj tasks (run as PE/DVE filler inside
            the next segment's attention). The per-head reciprocal row is
            broadcast across HD partitions by an SWDGE DMA (partition-step-0
            source AP) instead of a PE outer-product matmul."""
            if hc1 is None:
                hc1 = DC
            ctxu, ctx_s = st["ctxu"], st["ctx_s"]

            def norm_chunk(hc):
                # broadcast the HPC reciprocal rows of this head-chunk into a
                # full 128-partition tile (walrus requires equal base
                # partitions when both TensorTensor inputs are in SBUF)
                bcs = bcast_pool.tile([P, SEG], mm_dt, tag="bcs",
                                      name=f"bcs{s}_{hc}")
                rr = rc_dram[s:s + 1, hc * HPC * SEG:(hc + 1) * HPC * SEG]
                rr_b = bass.AP(tensor=rr.tensor, offset=rr.offset,
                               ap=[[SEG, HPC], [0, HD], [1, SEG]])
                nc.sync.dma_start(out=bcs, in_=rr_b)
                nc.vector.tensor_mul(ctx_s[:, hc, :], ctxu[:, hc, :], bcs)

            def outproj(fc):
                pso = pp_proj.tile([P, SEG], F32, tag="proj", name=f"pso{s}_{fc}")
                for dc in range(DC):
                    nc.tensor.matmul(
                        pso,
                        w_out_sb[:, dc, fc * P:(fc + 1) * P],
                        ctx_s[:, dc, :],
                        start=(dc == 0), stop=(dc == DC - 1))
                ot = out_pool.tile([P, SEG], mm_dt, tag="ot", name=f"ot{s}_{fc}")
                nc.vector.tensor_scalar_add(ot, pso, bout_sb[:, fc:fc + 1])
                nc.sync.dma_start(
                    out=outT[fc * P:(fc + 1) * P, s * SEG:(s + 1) * SEG], in_=ot)

            return ([(lambda hc=hc: norm_chunk(hc)) for hc in range(hc0, hc1)]
                    + [(lambda fc=fc: outproj(fc)) for fc in range(DC)]
                    if full else
                    [(lambda hc=hc: norm_chunk(hc)) for hc in range(hc0, hc1)])

        sts = {}
        SL = NSEG - 1
        for s in range(NSEG):
            sts[s] = load_and_proj(s)
            if s < SL:
                filler = list(norm_tasks(s - 1, sts.pop(s - 1))) if s > 0 else []
                attention(s, sts[s], filler)
                recip_chain(s, sts[s])

        # --- final segment: chunk order [1..5, 0]; recip + normalize +
        # partial outproj (chunks 1..5) all run as attention filler, so
        # after the last ctx only a 2-head recip + 1-chunk completion
        # remains ---
        stl = sts[SL]
        partial = ctxu_pool.tile([P, DC, SEG], mm_dt, tag="partial",
                                 name="partial", bufs=1)

        def partial_outproj(fc):
            pso = pp_proj.tile([P, SEG], F32, tag="proj", name=f"ppo{fc}")
            for dc in range(1, DC):
                nc.tensor.matmul(
                    pso,
                    w_out_sb[:, dc, fc * P:(fc + 1) * P],
                    stl["ctx_s"][:, dc, :],
                    start=(dc == 1), stop=(dc == DC - 1))
            nc.vector.tensor_scalar_add(partial[:, fc, :], pso,
                                        bout_sb[:, fc:fc + 1])

        prev_tasks = list(norm_tasks(SL - 1, sts.pop(SL - 1)))
        nrm = norm_tasks(SL, stl, hc0=1, hc1=DC, full=False)  # hc1..hc5 units
        # pin recip/normalize/partial units to iteration BOTTOMS so each is
        # emitted right after the ctx that produces its denominators (chunk
        # order [1..5,0]: ctx(cs[i-1]) lands at the bottom of iteration i)
        # masks for the endgame PE broadcast (K=1 matmuls): the head-h mask
        # column block replicates rc2's head-h row across its HD partitions
        mask2 = singles.tile([1, HPC * P], mm_dt, tag="mask2")
        nc.gpsimd.dma_start(out=mask2, in_=mask2_d[:, :])
        den2 = den_pool.tile([1, HPC * SEG], F32, tag="den2", bufs=1)
        rc2 = den_pool.tile([1, HPC * SEG], mm_dt, tag="rc2", bufs=1)

        filler_at = {
            2: [lambda: recip_chain(SL, stl, 2, 4)],
            3: [nrm[0], nrm[1]],
            4: [lambda: recip_chain(SL, stl, 6, 4)],
            5: [lambda: recip_chain(SL, stl, 10, 2), nrm[2], nrm[3]],
            6: ([nrm[4]]
                + [(lambda fc=fc: partial_outproj(fc)) for fc in range(DC)]),
        }
        attention(SL, stl, prev_tasks, cs=[1, 2, 3, 4, 5, 0],
                  filler_at=filler_at, den2_chunk=0, den2=den2)

        # endgame: heads 0,1 normalized without any DRAM hop — fast DVE
        # reciprocal on the [1,1024] row, bf16 cast, two K=1 PE broadcast
        # matmuls into PSUM, then the normalize mul
        rc2f = den_pool.tile([1, HPC * SEG], F32, tag="rc2f", bufs=1)
        nc.vector.reciprocal_approx_fast(rc2f, den2)
        with nc.allow_low_precision(
                reason="softmax denominator reciprocal; bf16 scale factor"):
            nc.vector.tensor_copy(rc2, rc2f)
        bcs_ps = pp_cb.tile([P, SEG], F32, tag="cb", name="bcsps")
        for hi in range(HPC):
            nc.tensor.matmul(bcs_ps, mask2[0:1, hi * P:(hi + 1) * P],
                             rc2[0:1, hi * SEG:(hi + 1) * SEG],
                             start=(hi == 0), stop=(hi == HPC - 1))
        nc.vector.tensor_mul(stl["ctx_s"][:, 0, :], stl["ctxu"][:, 0, :],
                             bcs_ps)
        for fc in range(DC):
            pso = pp_proj.tile([P, SEG], F32, tag="proj", name=f"cmp{fc}")
            nc.tensor.matmul(
                pso,
                w_out_sb[:, 0, fc * P:(fc + 1) * P],
                stl["ctx_s"][:, 0, :],
                start=True, stop=True)
            ot = out_pool.tile([P, SEG], mm_dt, tag="ot", name=f"eo{fc}")
            nc.vector.tensor_add(ot, partial[:, fc, :], pso)
            eng = nc.sync if fc % 2 == 0 else nc.scalar
            eng.dma_start(
                out=outT[fc * P:(fc + 1) * P, SL * SEG:(SL + 1) * SEG],
                in_=ot)

    nc.compile()
    return nc


def make_in_maps(x, Wqkv, bqkv, Wout, bout):
    """Shard full inputs across 8 cores: core = o*B + b."""
    r, E3, D = Wqkv.shape
    Bb, S, _ = x.shape
    DC = D // P
    in_maps = []
    for c in range(r * Bb):
        o, b = c // Bb, c % Bb
        W = np.ascontiguousarray(Wqkv[o])
        # ec-major q,k layout: row block g=sec*DC+ec is [128(p), DC*P] with
        # col dc*P+j = W[sec*D+ec*P+j, dc*P+p]
        wqk = W[:2 * D].reshape(2, DC, P, DC, P).transpose(0, 1, 4, 3, 2)
        wqkT_h = np.ascontiguousarray(wqk.reshape(2 * DC * P, D))
        hd = D // 12  # 64; 2 heads per 128-partition chunk
        hpc = P // hd
        mask2 = np.zeros((1, hpc * P), np.float32)
        for hi in range(hpc):
            mask2[0, hi * P + hi * hd: hi * P + (hi + 1) * hd] = 1.0
        in_maps.append({
            "xoT": np.ascontiguousarray(x[b, o::r, :].T).astype(ml_dtypes.bfloat16),
            "mask2": mask2.astype(ml_dtypes.bfloat16),
            "wqkT": wqkT_h.astype(ml_dtypes.bfloat16),
            "wvT": np.ascontiguousarray(W[2 * D:].T).astype(ml_dtypes.bfloat16),
            "woutT": np.ascontiguousarray(Wout[o].T).astype(ml_dtypes.bfloat16),
            "bqkv_pt": np.ascontiguousarray(bqkv[o].reshape(3 * DC, P).T),
            "bout_pt": np.ascontiguousarray(bout[o].reshape(DC, P).T),
            "bv": np.ascontiguousarray(bqkv[o, 2 * D:3 * D]),
        })
    return in_maps


_NC_CACHE = {}


def get_nc():
    if "nc" not in _NC_CACHE:
        _NC_CACHE["nc"] = build_nc()
    return _NC_CACHE["nc"]


def run(inputs, trace=False, **kwargs):
    """Run the SPMD kernel; returns (full_output, BassKernelResults)."""
    x = np.ascontiguousarray(np.asarray(inputs["x"], dtype=np.float32))
    Wqkv = np.asarray(inputs["Wqkv"], dtype=np.float32)
    bqkv = np.asarray(inputs["bqkv"], dtype=np.float32)
    Wout = np.asarray(inputs["Wout"], dtype=np.float32)
    bout = np.asarray(inputs["bout"], dtype=np.float32)
    r, E3, D = Wqkv.shape
    Bb, S, _ = x.shape

    nc = get_nc()
    in_maps = make_in_maps(x, Wqkv, bqkv, Wout, bout)
    res = run_bass_kernel_spmd(nc, in_maps, core_ids=list(range(len(in_maps))),
                               trace=trace, **kwargs)

    out = np.zeros((Bb, S, r * D), np.float32)
    for c in range(len(in_maps)):
        o, b = c // Bb, c % Bb
        out[b, o::r, o * D:(o + 1) * D] = \
            np.asarray(res.results[c]["outT"]).astype(np.float32).T
    return out, res


def kernel(x, Wqkv, bqkv, Wout, bout, num_heads):
    assert int(num_heads) == H0
    out, _ = run(dict(x=x, Wqkv=Wqkv, bqkv=bqkv, Wout=Wout, bout=bout))
    return out
